# revision 12
# baseline (speedup 1.0000x reference)
"""LoRA linear layer on 8 Trainium2 NeuronCores.

Computes y = x @ W^T + b + 2.0 * (x @ A^T) @ B^T for
x:[4,4096,1024], W:[1024,1024], b:[1024], A:[16,1024], B:[1024,16].

Host side folds the LoRA update into the weight (W_eff = W + 2*B@A, an exact
algebraic identity), so the device kernel is a single GEMM + bias. Sharding is
data-parallel over the 16384 tokens: each of the 8 cores computes a
[2048, 1024] output slice with replicated weights.

Device kernel (per core): y_c[m,o] = sum_d xT_c[d,m] * WeffT[d,o] + b[o]
  - x, Weff, y all bf16 on the wire (inputs quantized host-side; rel-err
    ~4e-3, well inside the 2e-2 gate), fp32 PSUM accumulation, f32 bias.
    Halves HBM traffic vs f32 so the kernel is purely PE-bound.
  - xT_c  [1024, 2048] bf16 (host-transposed so the contraction dim d lands
    on SBUF partitions for both matmul operands)
  - WeffT [1024, 1024] bf16, fully resident in SBUF
  - bf16 streams at 1 col/cycle like f32r, so the 256 N=512 matmuls floor at
    ~55 us; the rest of the schedule exists to keep head/tail off that path.
"""

import numpy as np
import ml_dtypes

import concourse.mybir as mybir
import concourse.tile as tile
from concourse import bacc
from concourse.bass_utils import run_bass_kernel_spmd

N_CORES = 8
P = 128
D = 1024  # in_features (contraction)
O = 1024  # out_features
M_TOTAL = 4 * 4096  # tokens
M = M_TOTAL // N_CORES  # tokens per core
KO = D // P  # k-subtiles
SC = 512  # m super-chunk
SCALING = 2.0

# Set by test harnesses to capture profiling info; harmless otherwise.
TRACE = False
LAST_RESULT = None

_NC_CACHE = None


def _build_nc():
    f32 = mybir.dt.float32
    bf16 = mybir.dt.bfloat16

    nc = bacc.Bacc("TRN2", debug=False)
    xT = nc.dram_tensor("xT", [D, M], bf16, kind="ExternalInput")
    wT = nc.dram_tensor("wT", [D, O], bf16, kind="ExternalInput")
    bias = nc.dram_tensor("bias", [P, O], f32, kind="ExternalInput")
    y = nc.dram_tensor("y", [M, O], bf16, kind="ExternalOutput")

    xT_v = xT[:].rearrange("(ko p) m -> p ko m", p=P)  # [128, 8, 2048]
    wT_v = wT[:].rearrange("(ko p) o -> p ko o", p=P)  # [128, 8, 1024]
    y_v = y[:].rearrange("(mt p) o -> p mt o", p=P)  # [128, 16, 1024]

    n_sc = M // SC
    MPC = SC // P  # m-tiles per super-chunk
    with tile.TileContext(nc) as tc:
        with (
            tc.tile_pool(name="wpool", bufs=1) as wpool,
            tc.tile_pool(name="bpool", bufs=1) as bpool,
            tc.tile_pool(name="x0pool", bufs=1) as x0pool,
            tc.tile_pool(name="xpool", bufs=3) as xpool,
            tc.tile_pool(name="opool", bufs=6) as opool,
            tc.tile_pool(name="psum", bufs=8, space="PSUM") as psum,
        ):
            # Head DMAs, latency-matched to when the half-waves consume them.
            # Each dma_start costs 600-830 ns of issue time on its engine, so
            # sc0/W arrive as few grouped transfers sized so granule ko lands
            # just before ko-round ko of the first wave: W half-0 slices as
            # ko0 | ko1-2 | ko3-7 on the Sync HWDGE ring, x0 likewise on the
            # Act HWDGE ring, then W half-1 as one 1 MiB transfer (not needed
            # until the second wave, ~7 us later).
            wt8 = wpool.tile([P, KO * O], bf16, tag="w")
            wt8_v = wt8[:].rearrange("p (ko o) -> p ko o", ko=KO)

            def wslice(ko, half):
                lo = ko * O + half * 512
                return wt8[:, lo : lo + 512]

            x0t = x0pool.tile([P, KO * SC], bf16, tag="x0")
            x0_g = xT_v[:, :, 0:SC]  # [128, 8, 512]

            def wload(k0, k1, half):
                lo, hi = half * 512, (half + 1) * 512
                nc.sync.dma_start(
                    wt8_v[:, k0:k1, lo:hi], wT_v[:, k0:k1, lo:hi]
                )

            def x0load(k0, k1):
                nc.scalar.dma_start(
                    x0t[:, k0 * SC : k1 * SC].rearrange(
                        "p (ko m) -> p ko m", ko=k1 - k0
                    ),
                    x0_g[:, k0:k1, :],
                )

            wload(0, 1, 0)
            x0load(0, 1)

            # PE warmup: ~30 N=128 matmuls on a zeroed tile span the HAM
            # clock-gate window (~3.4 us) while the first x/W slices stream
            # in, so real matmuls start warm (2.4 GHz) and fed.
            zt = bpool.tile([P, P], bf16, tag="warm")
            nc.gpsimd.memset(zt[:], 0.0)
            wps = psum.tile([P, 512], mybir.dt.float32, tag="ps", name="wps")
            for _ in range(30):
                nc.tensor.matmul(wps[:, :P], zt[:], zt[:], start=True, stop=True)

            wload(1, 3, 0)
            x0load(1, 3)
            wload(3, 8, 0)
            x0load(3, 8)
            wload(0, 8, 1)
            bt = bpool.tile([P, O], f32)
            nc.gpsimd.dma_start(bt[:], bias[:])

            # Later super-chunks arrive as one 1 MiB DMA each (same 1 KiB
            # per-partition runs as per-granule loads, 1/8 the instructions);
            # steady-state prefetch distance is a whole super-chunk (~14 us).
            xts = {}

            def load_x(sc, eng):
                t = xpool.tile([P, KO * SC], bf16, tag="xt", name=f"x{sc}")
                src = xT_v[:, :, sc * SC : (sc + 1) * SC]
                eng.dma_start(t[:].rearrange("p (ko m) -> p ko m", ko=KO), src)
                xts[sc] = t

            load_x(1, nc.scalar)

            def x_slice(sc, ko, mt_i):
                lo = mt_i * P
                if sc == 0:
                    return x0t[:, ko * SC + lo : ko * SC + lo + P]
                return xts[sc][:, ko * SC + lo : ko * SC + lo + P]

            def evict(ps, ot, o0, o1, p0=0):
                nc.vector.tensor_tensor(
                    ot[:, p0 : p0 + (o1 - o0)],
                    ps[:, 0 : o1 - o0],
                    bt[:, o0:o1],
                    mybir.AluOpType.add,
                )

            # Half-wave schedule: per super-chunk, first all eight ko-rounds
            # for the four half-0 PSUM groups (32 matmuls), then the half-1
            # groups. Only 4 PSUM banks are live per wave, so the other 4 are
            # free for the next wave and the DVE eviction burst (4 x 691 ns,
            # inside a ~7 us wave) never gates the PE. Interior stores ride
            # the idle SWDGE ring; the final wave special-cases the last
            # m-tile into two 256-wide groups so the closing eviction+store
            # chain is short.
            for sc in range(n_sc):
                if sc + 2 < n_sc:
                    load_x(sc + 2, nc.scalar)
                for half in range(2):
                    o0 = half * 512
                    final_wave = sc == n_sc - 1 and half == 1
                    if not final_wave:
                        pss = [
                            psum.tile(
                                [P, 512],
                                mybir.dt.float32,
                                tag="ps",
                                name=f"ps{sc}_{half}_{i}",
                            )
                            for i in range(MPC)
                        ]
                        for ko in range(KO):
                            for mt_i in range(MPC):
                                nc.tensor.matmul(
                                    pss[mt_i][:],
                                    x_slice(sc, ko, mt_i),
                                    wslice(ko, half),
                                    start=ko == 0,
                                    stop=ko == KO - 1,
                                )
                        for mt_i in range(MPC):
                            mt = sc * MPC + mt_i
                            ot = opool.tile(
                                [P, 512], bf16, tag="ot", name=f"ot{sc}_{half}_{mt_i}"
                            )
                            evict(pss[mt_i], ot, o0, o0 + 512)
                            eng = nc.gpsimd if sc < n_sc - 1 else nc.sync
                            eng.dma_start(
                                y_v[:, mt, o0 : o0 + 512], ot[:]
                            )
                    else:
                        # groups: mt0..2 full 512-wide, mt3 as two 256-wide
                        pss = [
                            psum.tile(
                                [P, 512], mybir.dt.float32, tag="ps", name=f"pf{i}"
                            )
                            for i in range(MPC - 1)
                        ]
                        # full-bank tiles (tag "ps" ring is exactly all of
                        # PSUM); only the first 256 columns are used
                        pq = [
                            psum.tile(
                                [P, 512], mybir.dt.float32, tag="ps", name=f"pq{q}"
                            )
                            for q in range(2)
                        ]
                        for ko in range(KO):
                            for mt_i in range(MPC - 1):
                                nc.tensor.matmul(
                                    pss[mt_i][:],
                                    x_slice(sc, ko, mt_i),
                                    wslice(ko, half),
                                    start=ko == 0,
                                    stop=ko == KO - 1,
                                )
                            for q in range(2):
                                nc.tensor.matmul(
                                    pq[q][:, 0:256],
                                    x_slice(sc, ko, MPC - 1),
                                    wt8[:, ko * O + o0 + q * 256 : ko * O + o0 + (q + 1) * 256],
                                    start=ko == 0,
                                    stop=ko == KO - 1,
                                )
                        for mt_i in range(MPC - 1):
                            mt = sc * MPC + mt_i
                            ot = opool.tile(
                                [P, 512], bf16, tag="ot", name=f"otf{mt_i}"
                            )
                            evict(pss[mt_i], ot, o0, o0 + 512)
                            nc.sync.dma_start(y_v[:, mt, o0 : o0 + 512], ot[:])
                        mt = sc * MPC + MPC - 1
                        otq = opool.tile([P, 512], bf16, tag="ot", name="otq")
                        for q in range(2):
                            q0 = o0 + q * 256
                            evict(pq[q], otq, q0, q0 + 256, p0=q * 256)
                            (nc.sync if q == 0 else nc.scalar).dma_start(
                                y_v[:, mt, q0 : q0 + 256],
                                otq[:, q * 256 : (q + 1) * 256],
                            )

    nc.compile()
    return nc


def _get_nc():
    global _NC_CACHE
    if _NC_CACHE is None:
        _NC_CACHE = _build_nc()
    return _NC_CACHE


def kernel(x, W, b, A, B):
    global LAST_RESULT
    x = np.ascontiguousarray(np.asarray(x, dtype=np.float32))
    W = np.asarray(W, dtype=np.float32)
    b = np.asarray(b, dtype=np.float32)
    A = np.asarray(A, dtype=np.float32)
    B = np.asarray(B, dtype=np.float32)
    assert x.shape == (4, 4096, D) and W.shape == (O, D)
    assert b.shape == (O,) and A.shape[1] == D and B.shape[0] == O

    # Fold the LoRA update into the weight: x@W^T + s*(x@A^T)@B^T = x@(W + s*B@A)^T
    Weff = (
        W.astype(np.float64) + SCALING * (B.astype(np.float64) @ A.astype(np.float64))
    ).astype(np.float32)
    WeffT = np.ascontiguousarray(Weff.T.astype(ml_dtypes.bfloat16))  # [D, O]
    bias_rep = np.ascontiguousarray(np.broadcast_to(b[None, :], (P, O)))

    xr = x.reshape(M_TOTAL, D).astype(ml_dtypes.bfloat16)
    in_maps = []
    for c in range(N_CORES):
        xTc = np.ascontiguousarray(xr[c * M : (c + 1) * M].T)  # [D, M]
        in_maps.append({"xT": xTc, "wT": WeffT, "bias": bias_rep})

    nc = _get_nc()
    res = run_bass_kernel_spmd(
        nc, in_maps, core_ids=list(range(N_CORES)), trace=TRACE
    )
    LAST_RESULT = res

    out = np.concatenate(
        [np.asarray(res.results[c]["y"]) for c in range(N_CORES)], axis=0
    )
    return out.astype(np.float32).reshape(x.shape[0], x.shape[1], O)


# revision 13
# speedup vs baseline: 1.0424x; 1.0424x over previous
"""LoRA linear layer on 8 Trainium2 NeuronCores.

Computes y = x @ W^T + b + 2.0 * (x @ A^T) @ B^T for
x:[4,4096,1024], W:[1024,1024], b:[1024], A:[16,1024], B:[1024,16].

Host side folds the LoRA update into the weight (W_eff = W + 2*B@A, an exact
algebraic identity), so the device kernel is a single GEMM + bias. Sharding is
data-parallel over the 16384 tokens: each of the 8 cores computes a
[2048, 1024] output slice with replicated weights.

Device kernel (per core): y_c[m,o] = sum_d xT_c[d,m] * WeffT[d,o] + b[o]
  - x, Weff, y all bf16 on the wire (rel-err ~3e-3, well inside the 2e-2
    gate), fp32 PSUM accumulation, f32 bias. bf16 streams 1 col/cycle like
    f32r, so the 131072 streamed columns floor at ~55 us of PE time; halving
    HBM traffic just keeps every byte off that critical path.
  - Host pre-tiles x/W/y so every DMA granule ([128, 512] x-slices, W
    half-slices, y output tiles) is one fully contiguous 128 KiB DRAM block:
    gappy 1 KiB-run patterns move at ~50-100 GB/s, contiguous at ~300+.
  - Head DMAs are grouped into a handful of transfers sized so granule ko
    lands just before the ko-round that consumes it (each dma_start costs
    ~0.7 us of issue time on its engine, so 16 tiny loads can't ramp fast
    enough; e2e latency is ~2 us + bytes/rate).
"""

import numpy as np
import ml_dtypes

import concourse.mybir as mybir
import concourse.tile as tile
from concourse import bacc
from concourse.bass_utils import run_bass_kernel_spmd

N_CORES = 8
P = 128
D = 1024  # in_features (contraction)
O = 1024  # out_features
M_TOTAL = 4 * 4096  # tokens
M = M_TOTAL // N_CORES  # tokens per core
KO = D // P  # k-subtiles
SC = 512  # m super-chunk
MT = M // P  # m-tiles per core (16)
SCALING = 2.0

# Set by test harnesses to capture profiling info; harmless otherwise.
TRACE = False
LAST_RESULT = None

_NC_CACHE = None


def _build_nc():
    f32 = mybir.dt.float32
    bf16 = mybir.dt.bfloat16

    nc = bacc.Bacc("TRN2", debug=False)
    # Host-tiled layouts: each leaf [128, 512] block is contiguous in DRAM.
    xT = nc.dram_tensor("xT", [(M // SC) * KO * P, SC], bf16, kind="ExternalInput")
    wT = nc.dram_tensor("wT", [KO * 2 * P, 512], bf16, kind="ExternalInput")
    bias = nc.dram_tensor("bias", [P, O], f32, kind="ExternalInput")
    y = nc.dram_tensor("y", [MT * 2 * P, 512], bf16, kind="ExternalOutput")

    x_v = xT[:].rearrange("(sc ko p) m -> p sc ko m", ko=KO, p=P)
    w_v = wT[:].rearrange("(kh p) o -> p kh o", p=P)  # kh = ko*2 + half
    y_v = y[:].rearrange("(mt h p) o -> p mt h o", h=2, p=P)

    n_sc = M // SC
    MPC = SC // P  # m-tiles per super-chunk
    with tile.TileContext(nc) as tc:
        with (
            tc.tile_pool(name="wpool", bufs=1) as wpool,
            tc.tile_pool(name="bpool", bufs=1) as bpool,
            tc.tile_pool(name="x0pool", bufs=1) as x0pool,
            tc.tile_pool(name="xpool", bufs=3) as xpool,
            tc.tile_pool(name="opool", bufs=8) as opool,
            tc.tile_pool(name="psum", bufs=8, space="PSUM") as psum,
        ):
            # W arrives as kh-groups on the Sync HWDGE ring, x's first
            # super-chunk as ko-groups on the Act HWDGE ring, each group
            # timed to beat the ko-round that first reads it.
            wt8 = wpool.tile([P, KO * 2 * 512], bf16, tag="w")

            def wslice(ko, half):
                lo = (ko * 2 + half) * 512
                return wt8[:, lo : lo + 512]

            def wload(s0, s1):
                nc.sync.dma_start(
                    wt8[:, s0 * 512 : s1 * 512].rearrange(
                        "p (kh o) -> p kh o", kh=s1 - s0
                    ),
                    w_v[:, s0:s1, :],
                )

            x0t = x0pool.tile([P, KO * SC], bf16, tag="x0")

            def x0load(k0, k1):
                nc.scalar.dma_start(
                    x0t[:, k0 * SC : k1 * SC].rearrange(
                        "p (ko m) -> p ko m", ko=k1 - k0
                    ),
                    x_v[:, 0, k0:k1, :],
                )

            wload(0, 1)
            x0load(0, 1)

            # PE warmup: N=128 matmuls on a zeroed tile span the HAM
            # clock-gate window (~3.4 us incl. the cold first real matmuls)
            # while the first x/W slices stream in.
            zt = bpool.tile([P, P], bf16, tag="warm")
            nc.gpsimd.memset(zt[:], 0.0)
            wps = psum.tile([P, 512], mybir.dt.float32, tag="ps", name="wps")
            for _ in range(24):
                nc.tensor.matmul(wps[:, :P], zt[:], zt[:], start=True, stop=True)

            wload(1, 2)
            x0load(1, 2)
            bt = bpool.tile([P, O], f32)
            nc.gpsimd.dma_start(bt[:], bias[:])
            wload(2, 4)
            x0load(2, 4)
            wload(4, 8)
            x0load(4, 8)
            wload(8, 16)

            # Later super-chunks arrive as one 1 MiB contiguous DMA each;
            # steady-state prefetch distance is a whole super-chunk (~14 us).
            xts = {}

            def load_x(sc):
                t = xpool.tile([P, KO * SC], bf16, tag="xt", name=f"x{sc}")
                nc.scalar.dma_start(
                    t[:].rearrange("p (ko m) -> p ko m", ko=KO),
                    x_v[:, sc, :, :],
                )
                xts[sc] = t

            load_x(1)

            def x_slice(sc, ko, mt_i):
                t = x0t if sc == 0 else xts[sc]
                lo = ko * SC + mt_i * P
                return t[:, lo : lo + P]

            def evict(ps, ot, mt, half, n):
                nc.vector.tensor_tensor(
                    ot[:, 0:n],
                    ps[:, 0:n],
                    bt[:, half * 512 : half * 512 + n],
                    mybir.AluOpType.add,
                )

            # Every super-chunk runs ko-outer: all four m-tiles x two halves
            # accumulate simultaneously across the 8 single-bank PSUM groups,
            # so each W/x slice is consumed as it lands during the ramp and
            # the PE never sits behind one large dependency. Evictions +
            # stores are inlined right behind each group's stop so PSUM slots
            # recycle smoothly into the next super-chunk.
            for sc in range(n_sc - 1):
                if sc + 2 < n_sc:
                    load_x(sc + 2)
                pss = [
                    [
                        psum.tile(
                            [P, 512], mybir.dt.float32, tag="ps", name=f"ps{sc}_{i}_{h}"
                        )
                        for h in range(2)
                    ]
                    for i in range(MPC)
                ]
                for ko in range(KO):
                    last = ko == KO - 1
                    for mt_i in range(MPC):
                        mt = sc * MPC + mt_i
                        for half in range(2):
                            nc.tensor.matmul(
                                pss[mt_i][half][:],
                                x_slice(sc, ko, mt_i),
                                wslice(ko, half),
                                start=ko == 0,
                                stop=last,
                            )
                        if last:
                            for half in range(2):
                                ot = opool.tile(
                                    [P, 512], bf16, tag="ot",
                                    name=f"ot{sc}_{mt_i}_{half}",
                                )
                                evict(pss[mt_i][half], ot, mt, half, 512)
                                nc.gpsimd.dma_start(y_v[:, mt, half, :], ot[:])

            # Last super-chunk: mt-outer, so evictions and stores spread
            # across its whole span instead of piling up after the final
            # matmul. The very last m-tile runs half 0 then half 1 as two
            # 256-wide quarter-groups, so the closing eviction+store chain
            # (what the end barrier waits on) is short, on otherwise-idle
            # queues.
            sc = n_sc - 1
            for mt_i in range(MPC):
                mt = sc * MPC + mt_i
                final = mt_i == MPC - 1
                if not final:
                    ph = [
                        psum.tile([P, 512], mybir.dt.float32, tag="ps", name=f"pl{h}")
                        for h in range(2)
                    ]
                    for ko in range(KO):
                        for half in range(2):
                            nc.tensor.matmul(
                                ph[half][:],
                                x_slice(sc, ko, mt_i),
                                wslice(ko, half),
                                start=ko == 0,
                                stop=ko == KO - 1,
                            )
                    for half in range(2):
                        ot = opool.tile(
                            [P, 512], bf16, tag="ot", name=f"otl{mt_i}_{half}"
                        )
                        evict(ph[half], ot, mt, half, 512)
                        nc.sync.dma_start(y_v[:, mt, half, :], ot[:])
                else:
                    ph0 = psum.tile([P, 512], mybir.dt.float32, tag="ps", name="pf0")
                    for ko in range(KO):
                        nc.tensor.matmul(
                            ph0[:],
                            x_slice(sc, ko, mt_i),
                            wslice(ko, 0),
                            start=ko == 0,
                            stop=ko == KO - 1,
                        )
                    ot0 = opool.tile([P, 512], bf16, tag="ot", name="otf0")
                    evict(ph0, ot0, mt, 0, 512)
                    nc.sync.dma_start(y_v[:, mt, 0, :], ot0[:])
                    # half 1 as two 256-wide quarter groups (full-bank tiles;
                    # the "ps" ring is exactly all of PSUM)
                    pq = [
                        psum.tile([P, 512], mybir.dt.float32, tag="ps", name=f"pq{q}")
                        for q in range(2)
                    ]
                    for q in range(2):
                        for ko in range(KO):
                            nc.tensor.matmul(
                                pq[q][:, 0:256],
                                x_slice(sc, ko, mt_i),
                                wslice(ko, 1)[:, q * 256 : (q + 1) * 256],
                                start=ko == 0,
                                stop=ko == KO - 1,
                            )
                    otq = opool.tile([P, 512], bf16, tag="ot", name="otq")
                    for q in range(2):
                        nc.vector.tensor_tensor(
                            otq[:, q * 256 : (q + 1) * 256],
                            pq[q][:, 0:256],
                            bt[:, 512 + q * 256 : 512 + (q + 1) * 256],
                            mybir.AluOpType.add,
                        )
                        (nc.sync if q == 0 else nc.scalar).dma_start(
                            y_v[:, mt, 1, q * 256 : (q + 1) * 256],
                            otq[:, q * 256 : (q + 1) * 256],
                        )

    nc.compile()
    return nc


def _get_nc():
    global _NC_CACHE
    if _NC_CACHE is None:
        _NC_CACHE = _build_nc()
    return _NC_CACHE


def kernel(x, W, b, A, B):
    global LAST_RESULT
    x = np.ascontiguousarray(np.asarray(x, dtype=np.float32))
    W = np.asarray(W, dtype=np.float32)
    b = np.asarray(b, dtype=np.float32)
    A = np.asarray(A, dtype=np.float32)
    B = np.asarray(B, dtype=np.float32)
    assert x.shape == (4, 4096, D) and W.shape == (O, D)
    assert b.shape == (O,) and A.shape[1] == D and B.shape[0] == O

    # Fold the LoRA update into the weight: x@W^T + s*(x@A^T)@B^T = x@(W + s*B@A)^T
    Weff = (
        W.astype(np.float64) + SCALING * (B.astype(np.float64) @ A.astype(np.float64))
    ).astype(np.float32)
    WeffT = Weff.T.astype(ml_dtypes.bfloat16)  # [D, O]
    # [KO, P, 2, 512] -> [KO, 2, P, 512]: leaf blocks contiguous per (ko, half)
    w_tiled = np.ascontiguousarray(
        WeffT.reshape(KO, P, 2, 512).transpose(0, 2, 1, 3)
    ).reshape(KO * 2 * P, 512)
    bias_rep = np.ascontiguousarray(np.broadcast_to(b[None, :], (P, O)))

    n_sc = M // SC
    xr = x.reshape(M_TOTAL, D).astype(ml_dtypes.bfloat16)
    in_maps = []
    for c in range(N_CORES):
        xc = xr[c * M : (c + 1) * M]  # [M, D]
        # x_t[sc, ko, p, j] = xc[sc*512 + j, ko*128 + p]
        x_tiled = np.ascontiguousarray(
            xc.reshape(n_sc, SC, KO, P).transpose(0, 2, 3, 1)
        ).reshape(n_sc * KO * P, SC)
        in_maps.append({"xT": x_tiled, "wT": w_tiled, "bias": bias_rep})

    nc = _get_nc()
    res = run_bass_kernel_spmd(
        nc, in_maps, core_ids=list(range(N_CORES)), trace=TRACE
    )
    LAST_RESULT = res

    outs = []
    for c in range(N_CORES):
        y_t = np.asarray(res.results[c]["y"]).reshape(MT, 2, P, 512)
        outs.append(y_t.transpose(0, 2, 1, 3).reshape(M, O))
    out = np.concatenate(outs, axis=0)
    return out.astype(np.float32).reshape(x.shape[0], x.shape[1], O)
